# revision 6
# baseline (speedup 1.0000x reference)
"""Causal self-attention on 8 Trainium2 NeuronCores.

Sharding: core = (batch b in {0,1}) x (head-group g in {0..3}), 4 heads per
core. Each core computes qkv for its heads from x[b], runs causal attention,
and multiplies by its 256 rows of w_proj, producing a partial [T, C] output
in bf16. Host sums the 4 partials per batch in f32.

Layout: everything is computed "transposed" so no on-chip transposes are
needed. The host feeds x[b].T in bf16; q^T/k^T come out of the qkv matmul
with head-dim on partitions (exactly the S^T = K Q^T operand layout); softmax
is done on S^T (keys on partitions, queries on free) with the denominator
obtained by appending a ones-column to V in the A@V matmul; the A@V output
Y^T is exactly the lhsT layout the final projection needs.

v3 perf notes (vs the 219us v1 / 202us v2):
- every DMA instruction costs ~600ns of issue time on its engine, so DMAs
  are merged (multi-c-tile input loads, one out DMA per row-tile, one ytr
  DMA per pair) and spread over two queues: x + out on sync, weights +
  eviction shuffles on gpsimd.
- causal masking is a PE matmul (identity lhsT x mask rhs accumulated into
  the S^T PSUM tile) instead of a DVE add: DVE PSUM ops run at 1 elem/cycle
  and sat on the S->exp critical chain.
- projection evictions are split scalar/vector so the latency-critical
  attention evictions don't queue behind them on DVE.
- query chunks run 3,2,1,0 so the final drain belongs to the cheapest
  chunk; normalization is per-pair; all HBM IO is bf16.
"""

import numpy as np
import ml_dtypes

import concourse.bass as bass
import concourse.bacc as bacc
import concourse.tile as tile
from concourse import mybir
from concourse.bass_utils import run_bass_kernel_spmd

F32 = mybir.dt.float32
BF16 = mybir.dt.bfloat16
EXP = mybir.ActivationFunctionType.Exp
COPY = mybir.ActivationFunctionType.Copy
BF16NP = ml_dtypes.bfloat16

B, T, C, H, HD = 2, 2048, 1024, 16, 64
NCORES = 8
HPC = 4      # heads per core
NPAIR = 2    # head pairs per core
NCT = C // 128   # 8 c-tiles
NTT = T // 128   # 16 t-tiles
NQC = T // 512   # 4 query chunks
SCALE = 1.0 / np.sqrt(HD)
NEG = -1.0e30


def build_kernel():
    nc = bacc.Bacc("TRN2", target_bir_lowering=False, debug=False, num_devices=NCORES)

    xT = nc.dram_tensor("xT", [C, T], BF16, kind="ExternalInput")
    wqk = nc.dram_tensor("wqk", [C, 512], BF16, kind="ExternalInput")
    wv = nc.dram_tensor("wv", [C, 256], BF16, kind="ExternalInput")
    wp = nc.dram_tensor("wp", [256, C], BF16, kind="ExternalInput")
    maskc = nc.dram_tensor("maskc", [128, 256], BF16, kind="ExternalInput")
    ident = nc.dram_tensor("ident", [128, 128], BF16, kind="ExternalInput")
    sel = nc.dram_tensor("sel", [2, 128], BF16, kind="ExternalInput")
    out = nc.dram_tensor("out", [T, C], BF16, kind="ExternalOutput")

    with tile.TileContext(nc) as tc:
        _body(tc, xT, wqk, wv, wp, maskc, ident, sel, out)

    nc.compile()
    return nc


def _body(tc, xT, wqk, wv, wp, maskc, ident, sel, out):
    nc = tc.nc
    from contextlib import ExitStack

    with ExitStack() as ctx:
        sb = lambda name: ctx.enter_context(tc.tile_pool(name=name, bufs=1))
        qkT_sb = sb("qkT").tile([128, 4 * T], BF16)       # bands q0,k0,q1,k1
        v65_sb = sb("v65").tile([128, NTT * 260], BF16)   # per k-tile: 4x(64 v + 1 ones)
        yt_sb = sb("yt").tile([128, NPAIR * T], BF16)     # pair p: rows 0-63 head 2p, 64-127 head 2p+1
        wp_sb = sb("wp").tile([128, 2 * C], BF16)
        maskc_sb = sb("maskc").tile([128, 256], BF16)
        ident_sb = sb("ident").tile([128, 128], BF16)
        sel_sb = sb("sel").tile([2, 128], BF16)

        es_pool = ctx.enter_context(tc.tile_pool(name="es", bufs=3))
        sums_pool = ctx.enter_context(tc.tile_pool(name="sums", bufs=2))
        rc_pool = ctx.enter_context(tc.tile_pool(name="rc", bufs=2))
        avst_pool = ctx.enter_context(tc.tile_pool(name="avst", bufs=2))
        ytr_pool = ctx.enter_context(tc.tile_pool(name="ytr", bufs=2))
        ost_pool = ctx.enter_context(tc.tile_pool(name="ost", bufs=3))

        # PSUM: psS tag = 2 slots x [128,1024] (4 banks), av 2 banks, misc 2
        ps = ctx.enter_context(tc.tile_pool(name="ps", bufs=2, space="PSUM"))
        av_pool = ctx.enter_context(tc.tile_pool(name="av", bufs=2, space="PSUM"))
        psS_pool = ctx.enter_context(tc.tile_pool(name="psS", bufs=2, space="PSUM"))

        # ones-columns of v65 (the softmax-denominator trick) via memset, not DMA
        v65_4d = v65_sb[:].rearrange("p (t h d) -> p t h d", t=NTT, h=HPC, d=65)
        nc.vector.memset(v65_4d[:, :, :, 64:65], 1.0)

        xw_pool = ctx.enter_context(tc.tile_pool(name="xw", bufs=1))
        xT_sb = xw_pool.tile([128, NCT * T], BF16, name="xT_sb")
        wqk_sb = xw_pool.tile([128, NCT * 512], BF16, name="wqk_sb")
        wv_sb = xw_pool.tile([128, NCT * 256], BF16, name="wv_sb")

        # input DMAs: few big instructions (each costs ~600ns issue time),
        # split over two queues, ordered so stage A's c-loop is paced by
        # arrival: x on sync, weights on gpsimd.
        def load_x(c0, ncx):
            dst = xT_sb[:, c0 * T:(c0 + ncx) * T].rearrange("p (c t) -> p c t", c=ncx)
            src = xT[c0 * 128:(c0 + ncx) * 128, :].rearrange("(c p) t -> p c t", c=ncx, p=128)
            nc.sync.dma_start(dst, src)

        def load_w(w, w_sb, cols, c0, ncx):
            dst = w_sb[:, c0 * cols:(c0 + ncx) * cols].rearrange("p (c w) -> p c w", c=ncx)
            src = w[c0 * 128:(c0 + ncx) * 128, :].rearrange("(c p) w -> p c w", c=ncx, p=128)
            nc.gpsimd.dma_start(dst, src)

        load_w(wqk, wqk_sb, 512, 0, 4)
        load_w(wv, wv_sb, 256, 0, 4)
        load_x(0, 1)
        load_x(1, 2)
        load_w(wqk, wqk_sb, 512, 4, 4)
        load_w(wv, wv_sb, 256, 4, 4)
        load_x(3, 2)
        load_x(5, 2)
        load_x(7, 1)
        nc.gpsimd.dma_start(
            wp_sb[:].rearrange("p (c w) -> p c w", c=2),
            wp.ap().rearrange("(c p) w -> p c w", c=2, p=128))
        nc.gpsimd.dma_start(maskc_sb[:], maskc[:])
        nc.gpsimd.dma_start(ident_sb[:], ident[:])
        nc.gpsimd.dma_start(sel_sb[:], sel[:])

        # ---- stage A: qkT bands [128, T] = w_band^T @ xT ----
        # c-outer over two-band passes: all 8 PSUM banks hold the 8
        # accumulating t-chunks, each c-tile of xT feeds 8 matmuls the
        # moment its DMA lands, and each lhsT load feeds 4 matmuls.
        for pass_ in range(2):
            bands = (2 * pass_, 2 * pass_ + 1)
            accs = {}
            for b in bands:
                acc01 = psS_pool.tile([128, 1024], F32, tag="psS", name=f"accA_{b}_01")
                acc2 = av_pool.tile([128, 512], F32, tag="av", name=f"accA_{b}_2")
                acc3 = ps.tile([128, 512], F32, tag="ps", name=f"accA_{b}_3")
                accs[b] = [acc01, acc2, acc3]
            for c in range(NCT):
                for b in bands:
                    lhs = wqk_sb[:, c * 512 + b * 128: c * 512 + (b + 1) * 128]
                    acc01, acc2, acc3 = accs[b]
                    dsts = [acc01[:, 0:512], acc01[:, 512:1024], acc2[:], acc3[:]]
                    for t4 in range(4):
                        nc.tensor.matmul(
                            dsts[t4], lhs,
                            xT_sb[:, c * T + t4 * 512: c * T + (t4 + 1) * 512],
                            start=(c == 0), stop=(c == NCT - 1))
            for b in bands:
                acc01, acc2, acc3 = accs[b]
                nc.vector.tensor_copy(qkT_sb[:, b * T: b * T + 1024], acc01[:])
                nc.vector.tensor_copy(qkT_sb[:, b * T + 1024: b * T + 1536], acc2[:])
                nc.vector.tensor_copy(qkT_sb[:, b * T + 1536: b * T + 2048], acc3[:])

        # ---- stage B: v natural [t, j]; tiles 0-3 now, the rest
        # drip-fed into stage C as PE filler (keeps HAM at full clock) --
        def emit_B(t):
            psv = ps.tile([128, 512], F32, tag="ps", name=f"psv_{t}")
            for c in range(NCT):
                lhs = xT_sb[:, c * T + t * 128: c * T + (t + 1) * 128]
                nc.tensor.matmul(psv[:, 0:256], lhs, wv_sb[:, c * 256:(c + 1) * 256],
                                 start=(c == 0), stop=(c == NCT - 1))
            dst = v65_sb[:, t * 260:(t + 1) * 260].rearrange("p (h d) -> p h d", h=HPC, d=65)
            src_ = psv[:, 0:256].rearrange("p (h d) -> p h d", h=HPC, d=64)
            nc.vector.tensor_copy(dst[:, :, 0:64], src_)

        for t in range(4):
            emit_B(t)

        # ---- stage C: attention; stage D: projection. All cross-chunk
        # serial work (normalization chain, projection) is drip-fed into the
        # NEXT chunk's kt loop so the in-order PE stream never stalls >3.4us
        # (a fully-idle HAM window would halve the PE clock for the rest of
        # the attention phase).
        def emit_proj_group(t):
            ost = ost_pool.tile([128, 1024], BF16, tag="ost", name=f"ost_{t}")
            for n in range(2):
                pso = ps.tile([128, 512], F32, tag="ps", name=f"pso_{t}_{n}")
                for p in range(NPAIR):
                    lhsT = yt_sb[:, p * T + t * 128: p * T + (t + 1) * 128]
                    rhs = wp_sb[:, p * C + n * 512: p * C + (n + 1) * 512]
                    nc.tensor.matmul(pso[:], lhsT, rhs, start=(p == 0), stop=(p == NPAIR - 1))
                if n == 0:
                    nc.scalar.activation(ost[:, 0:512], pso[:], COPY, scale=1.0)
                else:
                    nc.vector.tensor_copy(ost[:, 512:1024], pso[:])
            nc.sync.dma_start(out[t * 128:(t + 1) * 128, :], ost[:])

        pending = [(0, lambda t=t: emit_B(t)) for t in range(4, NTT)]

        for qc in (3, 2, 1, 0):
            nkt = 4 * qc + 4
            for p in range(NPAIR):
                qb, kb = 2 * p, 2 * p + 1
                av = [av_pool.tile([128, 512], F32, tag="av", name=f"av_{p}_{qc}_{i}") for i in range(2)]

                def emit_S(kt, p=p, qb=qb, kb=kb, qc=qc):
                    psb = psS_pool.tile([128, 1024], F32, tag="psS", name=f"psS_{p}_{qc}_{kt}")
                    d = kt - 4 * qc
                    slo = max(d, 0) * 128
                    for h in range(2):
                        base = 64 * h
                        lhsT = qkT_sb[base:base + 64, kb * T + kt * 128: kb * T + (kt + 1) * 128]
                        rhs = qkT_sb[base:base + 64, qb * T + qc * 512 + slo: qb * T + (qc + 1) * 512]
                        nc.tensor.matmul(psb[:, h * 512 + slo:(h + 1) * 512], lhsT, rhs,
                                         start=True, stop=(d < 0), tile_position=(base, 0))
                    if d >= 0:
                        # causal mask for the diagonal 128-col block, applied
                        # on the PE (identity lhsT) to keep DVE off this chain
                        for h in range(2):
                            nc.tensor.matmul(psb[:, h * 512 + slo: h * 512 + slo + 128],
                                             ident_sb[:],
                                             maskc_sb[:, h * 128:(h + 1) * 128],
                                             start=False, stop=True)
                    return psb

                pipe = [emit_S(0)]
                if nkt > 1:
                    pipe.append(emit_S(1))
                for kt in range(nkt):
                    cur = pipe.pop(0)
                    if kt + 2 < nkt:
                        pipe.append(emit_S(kt + 2))
                    d = kt - 4 * qc
                    lo = max(d, 0) * 128  # first valid query column of this k-tile
                    psb2 = cur[:].rearrange("p (h q) -> p h q", h=2, q=512)
                    es = es_pool.tile([128, 1024], BF16, tag="es", name=f"es_{p}_{qc}_{kt}")
                    es2 = es[:].rearrange("p (h q) -> p h q", h=2, q=512)
                    nc.scalar.activation(es2[:, :, lo:], psb2[:, :, lo:], EXP, scale=SCALE)
                    for h in range(2):
                        hh = 2 * p + h
                        lhsT_v = v65_sb[:, kt * 260 + hh * 65: kt * 260 + (hh + 1) * 65]
                        nc.tensor.matmul(av[h][0:65, lo:], lhsT_v, es[:, h * 512 + lo:(h + 1) * 512],
                                         start=(kt == 0), stop=(kt == nkt - 1))
                    popped = 0
                    while pending and pending[0][0] <= kt and popped < 3:
                        pending.pop(0)[1]()
                        popped += 1
                # evict Y^T + sums (PSUM can't feed DMA: stage via SBUF; the
                # partition shift for head b / sums rides the SBUF->SBUF DMA)
                ytr = ytr_pool.tile([128, 512], BF16, tag="ytr", name=f"ytr_{p}_{qc}")
                sums2 = sums_pool.tile([2, 512], BF16, tag="sums", name=f"sums_{p}_{qc}")
                st = avst_pool.tile([65, 1024], BF16, tag="avst", name=f"avst_{p}_{qc}")
                for h in range(2):
                    nc.vector.tensor_copy(st[:, h * 512:(h + 1) * 512], av[h][0:65, :])
                    nc.gpsimd.dma_start(ytr[64 * h:64 * (h + 1), :], st[0:64, h * 512:(h + 1) * 512])
                    nc.gpsimd.dma_start(sums2[h:h + 1, :], st[64:65, h * 512:(h + 1) * 512])

                def norm_pair(p=p, qc=qc, ytr=ytr, sums2=sums2):
                    rc2 = rc_pool.tile([2, 512], BF16, tag="rc", name=f"rc_{p}_{qc}")
                    with nc.allow_low_precision(reason="softmax denominators tolerate bf16"):
                        nc.vector.reciprocal(rc2[:], sums2[:])
                    psR = ps.tile([128, 512], F32, tag="ps", name=f"psR_{p}_{qc}")
                    nc.tensor.matmul(psR[:], sel_sb[:], rc2[:], start=True, stop=True)
                    nc.vector.tensor_mul(yt_sb[:, p * T + qc * 512: p * T + (qc + 1) * 512],
                                         ytr[:], psR[:])

                pending.append((2, norm_pair))

            pending += [(2, lambda t=t: emit_proj_group(t))
                        for t in range(4 * qc, 4 * qc + 4)]
        for _, fn in pending:
            fn()


_NC_CACHE = None


def _get_nc():
    global _NC_CACHE
    if _NC_CACHE is None:
        _NC_CACHE = build_kernel()
    return _NC_CACHE


def _make_in_maps(x, w_attn, w_proj):
    x = np.asarray(x, dtype=np.float32)
    w_attn = np.asarray(w_attn, dtype=np.float32)
    w_proj = np.asarray(w_proj, dtype=np.float32)
    # maskc: strictly-lower-triangular NEG blocks for the two heads of a pair
    # (row j = key, col i = query; masked iff j > i); pre-divided by SCALE
    # because the mask is added before the exp applies its scale.
    tri = np.tril(np.full((128, 128), NEG, dtype=np.float32), -1) / SCALE
    maskc = np.concatenate([tri, tri], axis=1)
    # sel broadcasts the per-head reciprocal row to that head's 64 partitions
    sel = np.zeros((2, 128), dtype=np.float32)
    sel[0, 0:64] = 1.0
    sel[1, 64:128] = 1.0
    ident = np.eye(128, dtype=np.float32)
    in_maps = []
    for core in range(NCORES):
        b, g = core // 4, core % 4
        hs = g * HPC
        q_cols = w_attn[:, hs * HD:(hs + HPC) * HD]
        k_cols = w_attn[:, C + hs * HD: C + (hs + HPC) * HD]
        v_cols = w_attn[:, 2 * C + hs * HD: 2 * C + (hs + HPC) * HD]
        wqk = np.concatenate(
            [q_cols[:, 0:128], k_cols[:, 0:128], q_cols[:, 128:256], k_cols[:, 128:256]], axis=1)
        in_maps.append({
            "xT": np.ascontiguousarray(x[b].T).astype(BF16NP),
            "wqk": np.ascontiguousarray(wqk).astype(BF16NP),
            "wv": np.ascontiguousarray(v_cols).astype(BF16NP),
            "wp": np.ascontiguousarray(w_proj[hs * HD:(hs + HPC) * HD, :]).astype(BF16NP),
            "maskc": maskc.astype(BF16NP),
            "ident": ident.astype(BF16NP),
            "sel": sel.astype(BF16NP),
        })
    return in_maps


def run_cores(x, w_attn, w_proj, trace=False):
    nc = _get_nc()
    in_maps = _make_in_maps(x, w_attn, w_proj)
    res = run_bass_kernel_spmd(nc, in_maps, core_ids=list(range(NCORES)), trace=trace)
    out = np.zeros((B, T, C), dtype=np.float32)
    for core in range(NCORES):
        out[core // 4] += np.asarray(res.results[core]["out"], dtype=np.float32)
    return out, res


def kernel(x, w_attn, w_proj):
    out, _ = run_cores(x, w_attn, w_proj, trace=False)
    return out


# revision 9
# speedup vs baseline: 1.1557x; 1.1557x over previous
"""Causal self-attention on 8 Trainium2 NeuronCores.

Sharding: core = (batch b in {0,1}) x (head-group g in {0..3}), 4 heads per
core. Each core computes qkv for its heads from x[b], runs causal attention,
and multiplies by its 256 rows of w_proj, producing a partial [T, C] output
in bf16. Host sums the 4 partials per batch in f32.

Layout: everything is computed "transposed" so no on-chip transposes are
needed. The host feeds x[b].T in bf16; q^T/k^T come out of the qkv matmul
with head-dim on partitions (exactly the S^T = K Q^T operand layout); softmax
is done on S^T (keys on partitions, queries on free) with the denominator
obtained by appending a ones-column to V in the A@V matmul; the A@V output
Y^T is exactly the lhsT layout the final projection needs.

v3 perf notes (vs the 219us v1 / 202us v2):
- every DMA instruction costs ~600ns of issue time on its engine, so DMAs
  are merged (multi-c-tile input loads, one out DMA per row-tile, one ytr
  DMA per pair) and spread over two queues: x + out on sync, weights +
  eviction shuffles on gpsimd.
- causal masking is a PE matmul (identity lhsT x mask rhs accumulated into
  the S^T PSUM tile) instead of a DVE add: DVE PSUM ops run at 1 elem/cycle
  and sat on the S->exp critical chain.
- projection evictions are split scalar/vector so the latency-critical
  attention evictions don't queue behind them on DVE.
- query chunks run 3,2,1,0 so the final drain belongs to the cheapest
  chunk; normalization is per-pair; all HBM IO is bf16.
"""

import numpy as np
import ml_dtypes

import concourse.bass as bass
import concourse.bacc as bacc
import concourse.tile as tile
from concourse import mybir
from concourse.bass_utils import run_bass_kernel_spmd

F32 = mybir.dt.float32
BF16 = mybir.dt.bfloat16
EXP = mybir.ActivationFunctionType.Exp
COPY = mybir.ActivationFunctionType.Copy
BF16NP = ml_dtypes.bfloat16

B, T, C, H, HD = 2, 2048, 1024, 16, 64
NCORES = 8
HPC = 4      # heads per core
NPAIR = 2    # head pairs per core
NCT = C // 128   # 8 c-tiles
NTT = T // 128   # 16 t-tiles
NQC = T // 512   # 4 query chunks
SCALE = 1.0 / np.sqrt(HD)
NEG = -1.0e30


def build_kernel():
    nc = bacc.Bacc("TRN2", target_bir_lowering=False, debug=False, num_devices=NCORES)

    xT = nc.dram_tensor("xT", [C, T], BF16, kind="ExternalInput")
    wqk = nc.dram_tensor("wqk", [C, 512], BF16, kind="ExternalInput")
    wv = nc.dram_tensor("wv", [C, 256], BF16, kind="ExternalInput")
    wp = nc.dram_tensor("wp", [256, C], BF16, kind="ExternalInput")
    maskc = nc.dram_tensor("maskc", [128, 256], BF16, kind="ExternalInput")
    ident = nc.dram_tensor("ident", [128, 128], BF16, kind="ExternalInput")
    sel = nc.dram_tensor("sel", [2, 128], BF16, kind="ExternalInput")
    out = nc.dram_tensor("out", [T, C], BF16, kind="ExternalOutput")

    with tile.TileContext(nc) as tc:
        _body(tc, xT, wqk, wv, wp, maskc, ident, sel, out)

    nc.compile()
    return nc


def _body(tc, xT, wqk, wv, wp, maskc, ident, sel, out):
    nc = tc.nc
    from contextlib import ExitStack

    with ExitStack() as ctx:
        sb = lambda name: ctx.enter_context(tc.tile_pool(name=name, bufs=1))
        qkT_sb = sb("qkT").tile([128, 4 * T], BF16)       # bands q0,k0,q1,k1
        v65_sb = sb("v65").tile([128, NTT * 260], BF16)   # per k-tile: 4x(64 v + 1 ones)
        yt_sb = sb("yt").tile([128, NPAIR * T], BF16)     # pair p: rows 0-63 head 2p, 64-127 head 2p+1
        wp_sb = sb("wp").tile([128, 2 * C], BF16)
        maskc_sb = sb("maskc").tile([128, 256], BF16)
        ident_sb = sb("ident").tile([128, 128], BF16)
        sel_sb = sb("sel").tile([2, 128], BF16)

        es_pool = ctx.enter_context(tc.tile_pool(name="es", bufs=3))
        sums_pool = ctx.enter_context(tc.tile_pool(name="sums", bufs=2))
        rc_pool = ctx.enter_context(tc.tile_pool(name="rc", bufs=2))
        avst_pool = ctx.enter_context(tc.tile_pool(name="avst", bufs=2))
        ytr_pool = ctx.enter_context(tc.tile_pool(name="ytr", bufs=2))
        ost_pool = ctx.enter_context(tc.tile_pool(name="ost", bufs=3))

        # PSUM: psS tag = 2 slots x [128,1024] (4 banks), av 2 banks, misc 2
        ps = ctx.enter_context(tc.tile_pool(name="ps", bufs=2, space="PSUM"))
        av_pool = ctx.enter_context(tc.tile_pool(name="av", bufs=2, space="PSUM"))
        psS_pool = ctx.enter_context(tc.tile_pool(name="psS", bufs=2, space="PSUM"))

        # ones-columns of v65 (the softmax-denominator trick) via memset, not DMA
        v65_4d = v65_sb[:].rearrange("p (t h d) -> p t h d", t=NTT, h=HPC, d=65)
        nc.vector.memset(v65_4d[:, :, :, 64:65], 1.0)

        xw_pool = ctx.enter_context(tc.tile_pool(name="xw", bufs=1))
        xT_sb = xw_pool.tile([128, NCT * T], BF16, name="xT_sb")
        wqk_sb = xw_pool.tile([128, NCT * 512], BF16, name="wqk_sb")
        wv_sb = xw_pool.tile([128, NCT * 256], BF16, name="wv_sb")

        # input DMAs: few big instructions (each costs ~600ns issue time),
        # split over two queues, ordered so stage A's c-loop is paced by
        # arrival: x on sync, weights on gpsimd.
        def load_x(c0, ncx):
            dst = xT_sb[:, c0 * T:(c0 + ncx) * T].rearrange("p (c t) -> p c t", c=ncx)
            src = xT[c0 * 128:(c0 + ncx) * 128, :].rearrange("(c p) t -> p c t", c=ncx, p=128)
            nc.sync.dma_start(dst, src)

        def load_w(w, w_sb, cols, c0, ncx):
            dst = w_sb[:, c0 * cols:(c0 + ncx) * cols].rearrange("p (c w) -> p c w", c=ncx)
            src = w[c0 * 128:(c0 + ncx) * 128, :].rearrange("(c p) w -> p c w", c=ncx, p=128)
            nc.gpsimd.dma_start(dst, src)

        load_w(wqk, wqk_sb, 512, 0, 1)
        load_w(wv, wv_sb, 256, 0, 1)
        load_x(0, 1)
        load_w(wqk, wqk_sb, 512, 1, 3)
        load_w(wv, wv_sb, 256, 1, 3)
        load_x(1, 2)
        load_w(wqk, wqk_sb, 512, 4, 4)
        load_w(wv, wv_sb, 256, 4, 4)
        load_x(3, 2)
        load_x(5, 2)
        load_x(7, 1)
        nc.gpsimd.dma_start(
            wp_sb[:].rearrange("p (c w) -> p c w", c=2),
            wp.ap().rearrange("(c p) w -> p c w", c=2, p=128))
        nc.gpsimd.dma_start(maskc_sb[:], maskc[:])
        nc.gpsimd.dma_start(ident_sb[:], ident[:])
        nc.gpsimd.dma_start(sel_sb[:], sel[:])

        # ---- stage A: qkT bands [128, T] = w_band^T @ xT ----
        # c-outer over two-band passes: all 8 PSUM banks hold the 8
        # accumulating t-chunks, each c-tile of xT feeds 8 matmuls the
        # moment its DMA lands, and each lhsT load feeds 4 matmuls.
        for pass_ in range(2):
            bands = (2 * pass_, 2 * pass_ + 1)
            accs = {}
            for b in bands:
                acc01 = psS_pool.tile([128, 1024], F32, tag="psS", name=f"accA_{b}_01")
                acc2 = av_pool.tile([128, 512], F32, tag="av", name=f"accA_{b}_2")
                acc3 = ps.tile([128, 512], F32, tag="ps", name=f"accA_{b}_3")
                accs[b] = [acc01, acc2, acc3]
            for c in range(NCT):
                for b in bands:
                    lhs = wqk_sb[:, c * 512 + b * 128: c * 512 + (b + 1) * 128]
                    acc01, acc2, acc3 = accs[b]
                    dsts = [acc01[:, 0:512], acc01[:, 512:1024], acc2[:], acc3[:]]
                    for t4 in range(4):
                        nc.tensor.matmul(
                            dsts[t4], lhs,
                            xT_sb[:, c * T + t4 * 512: c * T + (t4 + 1) * 512],
                            start=(c == 0), stop=(c == NCT - 1))
            for b in bands:
                acc01, acc2, acc3 = accs[b]
                nc.vector.tensor_copy(qkT_sb[:, b * T: b * T + 1024], acc01[:])
                nc.vector.tensor_copy(qkT_sb[:, b * T + 1024: b * T + 1536], acc2[:])
                nc.vector.tensor_copy(qkT_sb[:, b * T + 1536: b * T + 2048], acc3[:])

        # ---- stage B: v natural [t, j]; tiles 0-3 now, the rest
        # drip-fed into stage C as PE filler (keeps HAM at full clock) --
        def emit_B(t):
            psv = ps.tile([128, 512], F32, tag="ps", name=f"psv_{t}")
            for c in range(NCT):
                lhs = xT_sb[:, c * T + t * 128: c * T + (t + 1) * 128]
                nc.tensor.matmul(psv[:, 0:256], lhs, wv_sb[:, c * 256:(c + 1) * 256],
                                 start=(c == 0), stop=(c == NCT - 1))
            dst = v65_sb[:, t * 260:(t + 1) * 260].rearrange("p (h d) -> p h d", h=HPC, d=65)
            src_ = psv[:, 0:256].rearrange("p (h d) -> p h d", h=HPC, d=64)
            nc.vector.tensor_copy(dst[:, :, 0:64], src_)

        for t in range(4):
            emit_B(t)

        # ---- stage C: attention; stage D: projection. All cross-chunk
        # serial work (normalization chain, projection) is drip-fed into the
        # NEXT chunk's kt loop so the in-order PE stream never stalls >3.4us
        # (a fully-idle HAM window would halve the PE clock for the rest of
        # the attention phase).
        def emit_proj_group(t):
            ost = ost_pool.tile([128, 1024], BF16, tag="ost", name=f"ost_{t}")
            for n in range(2):
                pso = ps.tile([128, 512], F32, tag="ps", name=f"pso_{t}_{n}")
                for p in range(NPAIR):
                    lhsT = yt_sb[:, p * T + t * 128: p * T + (t + 1) * 128]
                    rhs = wp_sb[:, p * C + n * 512: p * C + (n + 1) * 512]
                    nc.tensor.matmul(pso[:], lhsT, rhs, start=(p == 0), stop=(p == NPAIR - 1))
                nc.vector.tensor_copy(ost[:, n * 512:(n + 1) * 512], pso[:])
            nc.sync.dma_start(out[t * 128:(t + 1) * 128, :], ost[:])

        pending = [(0, lambda t=t: emit_B(t)) for t in range(4, NTT)]

        for qc in (3, 2, 1, 0):
            nkt = 4 * qc + 4
            for p in range(NPAIR):
                qb, kb = 2 * p, 2 * p + 1
                av = [av_pool.tile([128, 512], F32, tag="av", name=f"av_{p}_{qc}_{i}") for i in range(2)]

                def emit_S(kt, p=p, qb=qb, kb=kb, qc=qc):
                    psb = psS_pool.tile([128, 1024], F32, tag="psS", name=f"psS_{p}_{qc}_{kt}")
                    d = kt - 4 * qc
                    slo = max(d, 0) * 128
                    for h in range(2):
                        base = 64 * h
                        lhsT = qkT_sb[base:base + 64, kb * T + kt * 128: kb * T + (kt + 1) * 128]
                        rhs = qkT_sb[base:base + 64, qb * T + qc * 512 + slo: qb * T + (qc + 1) * 512]
                        nc.tensor.matmul(psb[:, h * 512 + slo:(h + 1) * 512], lhsT, rhs,
                                         start=True, stop=(d < 0), tile_position=(base, 0))
                    if d >= 0:
                        # causal mask for the diagonal 128-col block, applied
                        # on the PE (identity lhsT) to keep DVE off this chain
                        for h in range(2):
                            nc.tensor.matmul(psb[:, h * 512 + slo: h * 512 + slo + 128],
                                             ident_sb[:],
                                             maskc_sb[:, h * 128:(h + 1) * 128],
                                             start=False, stop=True)
                    return psb

                pipe = [emit_S(0)]
                if nkt > 1:
                    pipe.append(emit_S(1))
                for kt in range(nkt):
                    cur = pipe.pop(0)
                    if kt + 2 < nkt:
                        pipe.append(emit_S(kt + 2))
                    d = kt - 4 * qc
                    lo = max(d, 0) * 128  # first valid query column of this k-tile
                    psb2 = cur[:].rearrange("p (h q) -> p h q", h=2, q=512)
                    es = es_pool.tile([128, 1024], BF16, tag="es", name=f"es_{p}_{qc}_{kt}")
                    es2 = es[:].rearrange("p (h q) -> p h q", h=2, q=512)
                    nc.scalar.activation(es2[:, :, lo:], psb2[:, :, lo:], EXP, scale=SCALE)
                    for h in range(2):
                        hh = 2 * p + h
                        lhsT_v = v65_sb[:, kt * 260 + hh * 65: kt * 260 + (hh + 1) * 65]
                        nc.tensor.matmul(av[h][0:65, lo:], lhsT_v, es[:, h * 512 + lo:(h + 1) * 512],
                                         start=(kt == 0), stop=(kt == nkt - 1))
                    popped = 0
                    while pending and pending[0][0] <= kt and popped < 3:
                        pending.pop(0)[1]()
                        popped += 1
                # evict Y^T + sums (PSUM can't feed DMA: stage via SBUF; the
                # partition shift for head b / sums rides the SBUF->SBUF DMA)
                ytr = ytr_pool.tile([128, 512], BF16, tag="ytr", name=f"ytr_{p}_{qc}")
                sums2 = sums_pool.tile([2, 512], F32, tag="sums", name=f"sums_{p}_{qc}")
                st = avst_pool.tile([65, 1024], BF16, tag="avst", name=f"avst_{p}_{qc}")
                for h in range(2):
                    nc.vector.tensor_copy(st[:, h * 512:(h + 1) * 512], av[h][0:65, :])
                    nc.gpsimd.dma_start(ytr[64 * h:64 * (h + 1), :], st[0:64, h * 512:(h + 1) * 512])
                    nc.gpsimd.dma_start(sums2[h:h + 1, :], st[64:65, h * 512:(h + 1) * 512])

                def norm_pair(p=p, qc=qc, ytr=ytr, sums2=sums2):
                    rcf = rc_pool.tile([2, 512], F32, tag="rcf", name=f"rcf_{p}_{qc}")
                    rc2 = rc_pool.tile([2, 512], BF16, tag="rc", name=f"rc_{p}_{qc}")
                    nc.vector.reciprocal_approx_fast(rcf[:], sums2[:])
                    nc.vector.tensor_copy(rc2[:], rcf[:])
                    psR = ps.tile([128, 512], F32, tag="ps", name=f"psR_{p}_{qc}")
                    nc.tensor.matmul(psR[:], sel_sb[:], rc2[:], start=True, stop=True)
                    nc.vector.tensor_mul(yt_sb[:, p * T + qc * 512: p * T + (qc + 1) * 512],
                                         ytr[:], psR[:])

                pending.append((2, norm_pair))

            pending += [(2, lambda t=t: emit_proj_group(t))
                        for t in range(4 * qc, 4 * qc + 4)]
        for _, fn in pending:
            fn()


_NC_CACHE = None


def _get_nc():
    global _NC_CACHE
    if _NC_CACHE is None:
        _NC_CACHE = build_kernel()
    return _NC_CACHE


def _make_in_maps(x, w_attn, w_proj):
    x = np.asarray(x, dtype=np.float32)
    w_attn = np.asarray(w_attn, dtype=np.float32)
    w_proj = np.asarray(w_proj, dtype=np.float32)
    # maskc: strictly-lower-triangular NEG blocks for the two heads of a pair
    # (row j = key, col i = query; masked iff j > i); pre-divided by SCALE
    # because the mask is added before the exp applies its scale.
    tri = np.tril(np.full((128, 128), NEG, dtype=np.float32), -1) / SCALE
    maskc = np.concatenate([tri, tri], axis=1)
    # sel broadcasts the per-head reciprocal row to that head's 64 partitions
    sel = np.zeros((2, 128), dtype=np.float32)
    sel[0, 0:64] = 1.0
    sel[1, 64:128] = 1.0
    ident = np.eye(128, dtype=np.float32)
    in_maps = []
    for core in range(NCORES):
        b, g = core // 4, core % 4
        hs = g * HPC
        q_cols = w_attn[:, hs * HD:(hs + HPC) * HD]
        k_cols = w_attn[:, C + hs * HD: C + (hs + HPC) * HD]
        v_cols = w_attn[:, 2 * C + hs * HD: 2 * C + (hs + HPC) * HD]
        wqk = np.concatenate(
            [q_cols[:, 0:128], k_cols[:, 0:128], q_cols[:, 128:256], k_cols[:, 128:256]], axis=1)
        in_maps.append({
            "xT": np.ascontiguousarray(x[b].T).astype(BF16NP),
            "wqk": np.ascontiguousarray(wqk).astype(BF16NP),
            "wv": np.ascontiguousarray(v_cols).astype(BF16NP),
            "wp": np.ascontiguousarray(w_proj[hs * HD:(hs + HPC) * HD, :]).astype(BF16NP),
            "maskc": maskc.astype(BF16NP),
            "ident": ident.astype(BF16NP),
            "sel": sel.astype(BF16NP),
        })
    return in_maps


def run_cores(x, w_attn, w_proj, trace=False):
    nc = _get_nc()
    in_maps = _make_in_maps(x, w_attn, w_proj)
    res = run_bass_kernel_spmd(nc, in_maps, core_ids=list(range(NCORES)), trace=trace)
    out = np.zeros((B, T, C), dtype=np.float32)
    for core in range(NCORES):
        out[core // 4] += np.asarray(res.results[core]["out"], dtype=np.float32)
    return out, res


def kernel(x, w_attn, w_proj):
    out, _ = run_cores(x, w_attn, w_proj, trace=False)
    return out


# revision 15
# speedup vs baseline: 1.2033x; 1.0412x over previous
"""Causal self-attention on 8 Trainium2 NeuronCores.

Sharding: core = (batch b in {0,1}) x (head-group g in {0..3}), 4 heads per
core. Each core computes qkv for its heads from x[b], runs causal attention,
and multiplies by its 256 rows of w_proj, producing a partial [T, C] output
in bf16. Host sums the 4 partials per batch in f32.

Layout: everything is computed "transposed" so no on-chip transposes are
needed. The host feeds x[b].T in bf16; q^T/k^T come out of the qkv matmul
with head-dim on partitions (exactly the S^T = K Q^T operand layout); softmax
is done on S^T (keys on partitions, queries on free) with the denominator
obtained by appending a ones-column to V in the A@V matmul; the A@V output
Y^T is exactly the lhsT layout the final projection needs.

v3 perf notes (vs the 219us v1 / 202us v2):
- every DMA instruction costs ~600ns of issue time on its engine, so DMAs
  are merged (multi-c-tile input loads, one out DMA per row-tile, one ytr
  DMA per pair) and spread over two queues: x + out on sync, weights +
  eviction shuffles on gpsimd.
- causal masking is a PE matmul (identity lhsT x mask rhs accumulated into
  the S^T PSUM tile) instead of a DVE add: DVE PSUM ops run at 1 elem/cycle
  and sat on the S->exp critical chain.
- projection evictions are split scalar/vector so the latency-critical
  attention evictions don't queue behind them on DVE.
- query chunks run 3,2,1,0 so the final drain belongs to the cheapest
  chunk; normalization is per-pair; all HBM IO is bf16.
"""

import numpy as np
import ml_dtypes

import concourse.bass as bass
import concourse.bacc as bacc
import concourse.tile as tile
from concourse import mybir
from concourse.bass_utils import run_bass_kernel_spmd

F32 = mybir.dt.float32
BF16 = mybir.dt.bfloat16
EXP = mybir.ActivationFunctionType.Exp
COPY = mybir.ActivationFunctionType.Copy
BF16NP = ml_dtypes.bfloat16

B, T, C, H, HD = 2, 2048, 1024, 16, 64
NCORES = 8
HPC = 4      # heads per core
NPAIR = 2    # head pairs per core
NCT = C // 128   # 8 c-tiles
NTT = T // 128   # 16 t-tiles
NQC = T // 512   # 4 query chunks
SCALE = 1.0 / np.sqrt(HD)
NEG = -1.0e30


def build_kernel():
    nc = bacc.Bacc("TRN2", target_bir_lowering=False, debug=False, num_devices=NCORES)

    xT = nc.dram_tensor("xT", [C, T], BF16, kind="ExternalInput")
    wqk = nc.dram_tensor("wqk", [C, 512], BF16, kind="ExternalInput")
    wv = nc.dram_tensor("wv", [C, 256], BF16, kind="ExternalInput")
    wp = nc.dram_tensor("wp", [256, C], BF16, kind="ExternalInput")
    maskc = nc.dram_tensor("maskc", [128, 256], BF16, kind="ExternalInput")
    ident = nc.dram_tensor("ident", [128, 128], BF16, kind="ExternalInput")
    sel = nc.dram_tensor("sel", [2, 128], BF16, kind="ExternalInput")
    out = nc.dram_tensor("out", [T, C], BF16, kind="ExternalOutput")

    with tile.TileContext(nc) as tc:
        _body(tc, xT, wqk, wv, wp, maskc, ident, sel, out)

    nc.compile()
    return nc


def _body(tc, xT, wqk, wv, wp, maskc, ident, sel, out):
    nc = tc.nc
    from contextlib import ExitStack

    with ExitStack() as ctx:
        sb = lambda name: ctx.enter_context(tc.tile_pool(name=name, bufs=1))
        qkT_sb = sb("qkT").tile([128, 4 * T], BF16)       # bands q0,k0,q1,k1
        v65_sb = sb("v65").tile([128, NTT * 260], BF16)   # per k-tile: 4x(64 v + 1 ones)
        yt_sb = sb("yt").tile([128, NPAIR * T], BF16)     # pair p: rows 0-63 head 2p, 64-127 head 2p+1
        wp_sb = sb("wp").tile([128, 2 * C], BF16)
        maskc_sb = sb("maskc").tile([128, 256], BF16)
        ident_sb = sb("ident").tile([128, 128], BF16)
        sel_sb = sb("sel").tile([2, 128], BF16)

        es_pool = ctx.enter_context(tc.tile_pool(name="es", bufs=3))
        sums_pool = ctx.enter_context(tc.tile_pool(name="sums", bufs=2))
        rc_pool = ctx.enter_context(tc.tile_pool(name="rc", bufs=2))
        avst_pool = ctx.enter_context(tc.tile_pool(name="avst", bufs=2))
        ytr_pool = ctx.enter_context(tc.tile_pool(name="ytr", bufs=2))
        ost_pool = ctx.enter_context(tc.tile_pool(name="ost", bufs=3))

        # PSUM: psS tag = 2 slots x [128,1024] (4 banks), av 2 banks, misc 2
        ps = ctx.enter_context(tc.tile_pool(name="ps", bufs=2, space="PSUM"))
        av_pool = ctx.enter_context(tc.tile_pool(name="av", bufs=2, space="PSUM"))
        psS_pool = ctx.enter_context(tc.tile_pool(name="psS", bufs=2, space="PSUM"))

        # ones-columns of v65 (the softmax-denominator trick) via memset, not DMA
        v65_4d = v65_sb[:].rearrange("p (t h d) -> p t h d", t=NTT, h=HPC, d=65)
        nc.vector.memset(v65_4d[:, :, :, 64:65], 1.0)

        xw_pool = ctx.enter_context(tc.tile_pool(name="xw", bufs=1))
        xT_sb = xw_pool.tile([128, NCT * T], BF16, name="xT_sb")
        wqk_sb = xw_pool.tile([128, NCT * 512], BF16, name="wqk_sb")
        wv_sb = xw_pool.tile([128, NCT * 256], BF16, name="wv_sb")

        # input DMAs: few big instructions (each costs ~600ns issue time)
        def load_w(w, w_sb, cols, c0, ncx):
            dst = w_sb[:, c0 * cols:(c0 + ncx) * cols].rearrange("p (c w) -> p c w", c=ncx)
            src = w[c0 * 128:(c0 + ncx) * 128, :].rearrange("(c p) w -> p c w", c=ncx, p=128)
            nc.gpsimd.dma_start(dst, src)

        # x even c-tiles on sync, odd on scalar (idle until the first exp
        # ~40us in); weights on gpsimd — three DGE pipelines ramp in
        # parallel so stage A's c-loop never starves.
        def load_x(c0, ncx, eng):
            dst = xT_sb[:, c0 * T:(c0 + ncx) * T].rearrange("p (c t) -> p c t", c=ncx)
            src = xT[c0 * 128:(c0 + ncx) * 128, :].rearrange("(c p) t -> p c t", c=ncx, p=128)
            eng.dma_start(dst, src)

        load_w(wqk, wqk_sb, 512, 0, 1)
        load_w(wv, wv_sb, 256, 0, 1)
        load_x(0, 1, nc.sync)
        load_x(1, 1, nc.scalar)
        load_w(wqk, wqk_sb, 512, 1, 3)
        load_w(wv, wv_sb, 256, 1, 3)
        load_x(2, 1, nc.sync)
        load_x(3, 1, nc.scalar)
        load_w(wqk, wqk_sb, 512, 4, 4)
        load_w(wv, wv_sb, 256, 4, 4)
        load_x(4, 1, nc.sync)
        load_x(5, 1, nc.scalar)
        load_x(6, 1, nc.sync)
        load_x(7, 1, nc.scalar)
        nc.gpsimd.dma_start(
            wp_sb[:].rearrange("p (c w) -> p c w", c=2),
            wp.ap().rearrange("(c p) w -> p c w", c=2, p=128))
        nc.gpsimd.dma_start(maskc_sb[:], maskc[:])
        nc.gpsimd.dma_start(ident_sb[:], ident[:])
        nc.gpsimd.dma_start(sel_sb[:], sel[:])

        # ---- stage A: qkT bands [128, T] = w_band^T @ xT ----
        # c-outer over two-band passes: all 8 PSUM banks hold the 8
        # accumulating t-chunks, each c-tile of xT feeds 8 matmuls the
        # moment its DMA lands, and each lhsT load feeds 4 matmuls.
        for pass_ in range(2):
            bands = (2 * pass_, 2 * pass_ + 1)
            accs = {}
            for b in bands:
                acc01 = psS_pool.tile([128, 1024], F32, tag="psS", name=f"accA_{b}_01")
                acc2 = av_pool.tile([128, 512], F32, tag="av", name=f"accA_{b}_2")
                acc3 = ps.tile([128, 512], F32, tag="ps", name=f"accA_{b}_3")
                accs[b] = [acc01, acc2, acc3]
            for c in range(NCT):
                for b in bands:
                    lhs = wqk_sb[:, c * 512 + b * 128: c * 512 + (b + 1) * 128]
                    acc01, acc2, acc3 = accs[b]
                    dsts = [acc01[:, 0:512], acc01[:, 512:1024], acc2[:], acc3[:]]
                    for t4 in range(4):
                        nc.tensor.matmul(
                            dsts[t4], lhs,
                            xT_sb[:, c * T + t4 * 512: c * T + (t4 + 1) * 512],
                            start=(c == 0), stop=(c == NCT - 1))
            for b in bands:
                acc01, acc2, acc3 = accs[b]
                nc.vector.tensor_copy(qkT_sb[:, b * T: b * T + 1024], acc01[:])
                nc.vector.tensor_copy(qkT_sb[:, b * T + 1024: b * T + 1536], acc2[:])
                nc.vector.tensor_copy(qkT_sb[:, b * T + 1536: b * T + 2048], acc3[:])

        # ---- stage B: v natural [t, j]; tiles 0-3 now, the rest
        # drip-fed into stage C as PE filler (keeps HAM at full clock) --
        def emit_B(t):
            psv = ps.tile([128, 512], F32, tag="ps", name=f"psv_{t}")
            for c in range(NCT):
                lhs = xT_sb[:, c * T + t * 128: c * T + (t + 1) * 128]
                nc.tensor.matmul(psv[:, 0:256], lhs, wv_sb[:, c * 256:(c + 1) * 256],
                                 start=(c == 0), stop=(c == NCT - 1))
            dst = v65_sb[:, t * 260:(t + 1) * 260].rearrange("p (h d) -> p h d", h=HPC, d=65)
            src_ = psv[:, 0:256].rearrange("p (h d) -> p h d", h=HPC, d=64)
            nc.vector.tensor_copy(dst[:, :, 0:64], src_)

        for t in range(2):
            emit_B(t)

        # ---- stage C: attention; stage D: projection. All cross-chunk
        # serial work (normalization chain, projection) is drip-fed into the
        # NEXT chunk's kt loop so the in-order PE stream never stalls >3.4us
        # (a fully-idle HAM window would halve the PE clock for the rest of
        # the attention phase).
        def emit_proj_group(t):
            ost = ost_pool.tile([128, 1024], BF16, tag="ost", name=f"ost_{t}")
            for n in range(2):
                pso = ps.tile([128, 512], F32, tag="ps", name=f"pso_{t}_{n}")
                for p in range(NPAIR):
                    lhsT = yt_sb[:, p * T + t * 128: p * T + (t + 1) * 128]
                    rhs = wp_sb[:, p * C + n * 512: p * C + (n + 1) * 512]
                    nc.tensor.matmul(pso[:], lhsT, rhs, start=(p == 0), stop=(p == NPAIR - 1))
                nc.vector.tensor_copy(ost[:, n * 512:(n + 1) * 512], pso[:])
            nc.sync.dma_start(out[t * 128:(t + 1) * 128, :], ost[:])

        pending = [(0, lambda t=t: emit_B(t)) for t in range(2, NTT)]

        for qc in (0, 1, 2, 3):
            nkt = 4 * qc + 4
            for p in range(NPAIR):
                qb, kb = 2 * p, 2 * p + 1
                av = [av_pool.tile([128, 512], F32, tag="av", name=f"av_{p}_{qc}_{i}") for i in range(2)]

                def emit_S(kt, p=p, qb=qb, kb=kb, qc=qc):
                    psb = psS_pool.tile([128, 1024], F32, tag="psS", name=f"psS_{p}_{qc}_{kt}")
                    d = kt - 4 * qc
                    slo = max(d, 0) * 128
                    for h in range(2):
                        base = 64 * h
                        lhsT = qkT_sb[base:base + 64, kb * T + kt * 128: kb * T + (kt + 1) * 128]
                        rhs = qkT_sb[base:base + 64, qb * T + qc * 512 + slo: qb * T + (qc + 1) * 512]
                        nc.tensor.matmul(psb[:, h * 512 + slo:(h + 1) * 512], lhsT, rhs,
                                         start=True, stop=(d < 0), tile_position=(base, 0))
                    if d >= 0:
                        # causal mask for the diagonal 128-col block, applied
                        # on the PE (identity lhsT) to keep DVE off this chain
                        for h in range(2):
                            nc.tensor.matmul(psb[:, h * 512 + slo: h * 512 + slo + 128],
                                             ident_sb[:],
                                             maskc_sb[:, h * 128:(h + 1) * 128],
                                             start=False, stop=True)
                    return psb

                pipe = [emit_S(0)]
                if nkt > 1:
                    pipe.append(emit_S(1))
                for kt in range(nkt):
                    cur = pipe.pop(0)
                    if kt + 2 < nkt:
                        pipe.append(emit_S(kt + 2))
                    d = kt - 4 * qc
                    lo = max(d, 0) * 128  # first valid query column of this k-tile
                    psb2 = cur[:].rearrange("p (h q) -> p h q", h=2, q=512)
                    es = es_pool.tile([128, 1024], BF16, tag="es", name=f"es_{p}_{qc}_{kt}")
                    es2 = es[:].rearrange("p (h q) -> p h q", h=2, q=512)
                    nc.scalar.activation(es2[:, :, lo:], psb2[:, :, lo:], EXP, scale=SCALE)
                    for h in range(2):
                        hh = 2 * p + h
                        lhsT_v = v65_sb[:, kt * 260 + hh * 65: kt * 260 + (hh + 1) * 65]
                        nc.tensor.matmul(av[h][0:65, lo:], lhsT_v, es[:, h * 512 + lo:(h + 1) * 512],
                                         start=(kt == 0), stop=(kt == nkt - 1))
                    popped = 0
                    while pending and pending[0][0] <= kt and popped < 2:
                        pending.pop(0)[1]()
                        popped += 1
                # evict Y^T + sums (PSUM can't feed DMA: stage via SBUF; the
                # partition shift for head b / sums rides the SBUF->SBUF DMA)
                ytr = ytr_pool.tile([128, 512], BF16, tag="ytr", name=f"ytr_{p}_{qc}")
                sums2 = sums_pool.tile([2, 512], F32, tag="sums", name=f"sums_{p}_{qc}")
                st = avst_pool.tile([65, 1024], BF16, tag="avst", name=f"avst_{p}_{qc}")
                for h in range(2):
                    nc.vector.tensor_copy(st[:, h * 512:(h + 1) * 512], av[h][0:65, :])
                    nc.gpsimd.dma_start(ytr[64 * h:64 * (h + 1), :], st[0:64, h * 512:(h + 1) * 512])
                    nc.gpsimd.dma_start(sums2[h:h + 1, :], st[64:65, h * 512:(h + 1) * 512])

                def norm_pair(p=p, qc=qc, ytr=ytr, sums2=sums2):
                    rcf = rc_pool.tile([2, 512], F32, tag="rcf", name=f"rcf_{p}_{qc}")
                    rc2 = rc_pool.tile([2, 512], BF16, tag="rc", name=f"rc_{p}_{qc}")
                    nc.vector.reciprocal_approx_fast(rcf[:], sums2[:])
                    nc.vector.tensor_copy(rc2[:], rcf[:])
                    psR = ps.tile([128, 512], F32, tag="ps", name=f"psR_{p}_{qc}")
                    nc.tensor.matmul(psR[:], sel_sb[:], rc2[:], start=True, stop=True)
                    nc.vector.tensor_mul(yt_sb[:, p * T + qc * 512: p * T + (qc + 1) * 512],
                                         ytr[:], psR[:])

                pending.append((2, norm_pair))

            pending += [(2, lambda t=t: emit_proj_group(t))
                        for t in range(4 * qc, 4 * qc + 4)]
        for _, fn in pending:
            fn()


_NC_CACHE = None


def _get_nc():
    global _NC_CACHE
    if _NC_CACHE is None:
        _NC_CACHE = build_kernel()
    return _NC_CACHE


def _make_in_maps(x, w_attn, w_proj):
    x = np.asarray(x, dtype=np.float32)
    w_attn = np.asarray(w_attn, dtype=np.float32)
    w_proj = np.asarray(w_proj, dtype=np.float32)
    # maskc: strictly-lower-triangular NEG blocks for the two heads of a pair
    # (row j = key, col i = query; masked iff j > i); pre-divided by SCALE
    # because the mask is added before the exp applies its scale.
    tri = np.tril(np.full((128, 128), NEG, dtype=np.float32), -1) / SCALE
    maskc = np.concatenate([tri, tri], axis=1)
    # sel broadcasts the per-head reciprocal row to that head's 64 partitions
    sel = np.zeros((2, 128), dtype=np.float32)
    sel[0, 0:64] = 1.0
    sel[1, 64:128] = 1.0
    ident = np.eye(128, dtype=np.float32)
    in_maps = []
    for core in range(NCORES):
        b, g = core // 4, core % 4
        hs = g * HPC
        q_cols = w_attn[:, hs * HD:(hs + HPC) * HD]
        k_cols = w_attn[:, C + hs * HD: C + (hs + HPC) * HD]
        v_cols = w_attn[:, 2 * C + hs * HD: 2 * C + (hs + HPC) * HD]
        wqk = np.concatenate(
            [q_cols[:, 0:128], k_cols[:, 0:128], q_cols[:, 128:256], k_cols[:, 128:256]], axis=1)
        in_maps.append({
            "xT": np.ascontiguousarray(x[b].T).astype(BF16NP),
            "wqk": np.ascontiguousarray(wqk).astype(BF16NP),
            "wv": np.ascontiguousarray(v_cols).astype(BF16NP),
            "wp": np.ascontiguousarray(w_proj[hs * HD:(hs + HPC) * HD, :]).astype(BF16NP),
            "maskc": maskc.astype(BF16NP),
            "ident": ident.astype(BF16NP),
            "sel": sel.astype(BF16NP),
        })
    return in_maps


def run_cores(x, w_attn, w_proj, trace=False):
    nc = _get_nc()
    in_maps = _make_in_maps(x, w_attn, w_proj)
    res = run_bass_kernel_spmd(nc, in_maps, core_ids=list(range(NCORES)), trace=trace)
    out = np.zeros((B, T, C), dtype=np.float32)
    for core in range(NCORES):
        out[core // 4] += np.asarray(res.results[core]["out"], dtype=np.float32)
    return out, res


def kernel(x, w_attn, w_proj):
    out, _ = run_cores(x, w_attn, w_proj, trace=False)
    return out
